# revision 12
# baseline (speedup 1.0000x reference)
"""Bilateral denoiser (11x11 window) on 8 Trainium2 NeuronCores.

Sharding: 8 cores = 4 images x 2 column halves (512 cols each, full 1024 rows).
Device layout: each of 128 partitions owns a 32x64 pixel block with a 5-pixel
halo stored in the free dimension, so both dy and dx taps are free-dim offsets.
Two mega-tiles per core cover rows [0,512) and [512,1024).  The host
pre-gathers each partition's halo'd block so all DMAs are contiguous 2D.

Weight math per tap t=(dy,dx), d=|t|:
  w_ref = exp(-d^2/8) * clip(n.t_n,0,1)^128 * exp(-|t_z-z|/max(dz*d,1e-4))
computed as   s = ln(clip(dot,1e-30,1)) - |(t_z-z) * r'|,  w = exp(128*s - d^2/8)
with r' = min(recip_dz/(128 d), 1e4/128).  Mirror taps (-t) reuse ln(dot) of
(+t) shifted, since dot is symmetric.  Out-of-image pixels are padded
host-side with z=1e18 (forces w=0) and 0 for the other planes.
"""

import sys

for _p in ("/root/.axon_site", "/root/.axon_site/_ro/trn_rl_repo",
           "/root/.axon_site/_ro/pypackages", "/opt/trn_rl_repo", "/opt/pypackages"):
    if _p not in sys.path:
        sys.path.append(_p)

import math
import numpy as np

B, H, W = 4, 1024, 1024
RAD = 5
NCORES = 8
COLS = 512          # cols per core
RB, CW = 32, 64     # block rows/cols per partition
HB, HC = RB + 2 * RAD, CW + 2 * RAD          # 42, 74  halo'd block
EB, EC = RB + RAD, CW + 2 * RAD              # 37, 74  expanded (dot/ln) frame
NTILES = 2          # mega-tiles (rows 0..512, 512..1024)
GR, GC = 16, 8      # partition grid: 16 row-blocks x 8 col-blocks
PR, PC = H + 2 * RAD, COLS + 2 * RAD         # 1034 x 522 padded plane dims
ZPAD = 1.0e18
CLIP_LO = 1.0e-30
RCAP = 1.0e4 / 128.0
USE_BCAST = True  # step-0 broadcast APs on DVE inputs
NPIX = RB * CW      # 2048
NSH = HB * HC       # 3108 halo'd plane elems/partition
NEX = EB * EC       # 2738 expanded plane elems/partition

# plane order in the shifted-plane bundle
PNX, PNY, PNZ, PCR, PCG, PCB, PZ = range(7)

_cache = {}


def _positive_half_taps():
    taps = []
    for dy in range(0, RAD + 1):
        for dx in range(-RAD, RAD + 1):
            if dy == 0 and dx <= 0:
                continue
            taps.append((dy, dx))
    # group by d^2 so rprime is computed once per distinct distance
    taps.sort(key=lambda t: (t[0] * t[0] + t[1] * t[1], t))
    return taps


def _build_program(ntiles=NTILES, npairs=None):
    import concourse.bass as bass
    import concourse.mybir as mybir
    import concourse.tile as tile

    f32 = mybir.dt.float32
    Alu = mybir.AluOpType
    Act = mybir.ActivationFunctionType

    # --- workaround: this walrus build rejects >1 sync-wait per TPB_CTRL ---
    def _patched_drain_and_barrier(self, tick_clock, wait_clock):
        from bass_rust import ScopedClock

        probe = mybir.InstDrain(
            name="wait-probe", engine=mybir.EngineType.SP, ins=[], outs=[]
        )
        wait_clock.add_sem_waits(probe, ScopedClock({None: tick_clock.global_clock}))
        si = probe.sync_info
        waits = list(si.on_wait) if si is not None else []
        handles = {h.num: h for h in self.sems.allocated().values()}
        for wt in waits:
            assert wt.wait_reg is None and wt.wait_mode == "sem-ge-imm", wt
            self.nc.sync.wait_ge(handles[wt.id], wt.wait_value)
        self.nc.sync.drain()
        self.nc.all_engine_barrier()
        popped = self.nc._tile_sem_poison_stack.pop()
        assert popped is self._sem_poison
        self.nc.clear_and_free_semaphores(list(self.sems.allocated().values()))
        self.nc.all_engine_barrier()

    tile.TileContext._drain_and_barrier = _patched_drain_and_barrier

    def _split_multi_waits(nc):
        # This walrus build accepts only one sync-wait per instruction;
        # hoist extra waits onto NoOps inserted just before, same engine.
        total = 0
        for blk in nc.main_func.blocks:
            il = blk.instructions
            k = 0
            while k < len(il):
                inst = il[k]
                si = inst.sync_info
                if si is not None and si.on_wait and len(si.on_wait) > 1:
                    waits = list(si.on_wait)
                    for j, wt in enumerate(waits[:-1]):
                        nop = mybir.InstNoOp(
                            name=f"{inst.name}-hw{j}",
                            engine=inst.engine,
                            ins=[],
                            outs=[],
                        )
                        nop.sync_info = mybir.SyncInfo(on_wait=[wt], on_update=[])
                        nc.register_instruction(nop, overwrite=True)
                        il.insert(k, nop)
                        k += 1
                    inst.sync_info = mybir.SyncInfo(
                        on_wait=[waits[-1]], on_update=list(si.on_update or [])
                    )
                    total += 1
                k += 1
        return total

    nc = bass.Bass("TRN2")

    taps = _positive_half_taps()
    if npairs is not None:
        taps = taps[:npairs]
    dsqs = sorted({dy * dy + dx * dx for dy, dx in taps})
    for d2 in dsqs:
        v = -(d2) / 8.0
        key = (f32, float(v))
        if key not in nc.const_aps.aps:
            t = nc.alloc_sbuf_tensor(f"cbias_{d2}", [128, 1], f32)
            nc.gpsimd.memset(t.ap(), float(v))
            nc.const_aps.aps[key] = t.ap()
    nc.all_engine_barrier()

    shp_t = nc.dram_tensor("shp", [ntiles, 128, 7 * NSH], f32, kind="ExternalInput")
    dz_t = nc.dram_tensor("dz", [ntiles, 128, NPIX], f32, kind="ExternalInput")
    out_t = nc.dram_tensor("out", [ntiles, 128, 3 * NPIX], f32, kind="ExternalOutput")

    def mkap(base, dims, extra_off):
        a = base.copy()
        a.ap = mybir.VecI64Pair(dims)
        a.offset = a.offset + extra_off
        return a

    with tile.TileContext(nc) as tc:
        with tc.tile_pool(name="shp", bufs=1) as shp_pool, \
             tc.tile_pool(name="exp", bufs=1) as exp_pool, \
             tc.tile_pool(name="cen", bufs=1) as cen_pool, \
             tc.tile_pool(name="rot", bufs=2) as rot_pool, \
             tc.tile_pool(name="pw", bufs=2, space="PSUM") as psum_pool:
            for T in range(ntiles):
                shp = shp_pool.tile([128, 7 * NSH], f32, tag="shp")
                ex = exp_pool.tile([128, 3 * NEX], f32, tag="ex")
                acc = cen_pool.tile([128, 4 * NPIX], f32, tag="acc")
                recip = cen_pool.tile([128, NPIX], f32, tag="recip")
                rprime = cen_pool.tile([128, NPIX], f32, tag="rprime")
                dl3 = cen_pool.tile([128, 3 * NPIX], f32, tag="dl3")

                def hview(k, y0, ny, x0, nx, _shp=shp):
                    # halo'd plane k at block coords rows y0.., cols x0..
                    # (block-local pixel coords; -5 = first halo row/col)
                    ps = _shp.ap[0][0]
                    off = k * NSH + (y0 + RAD) * HC + (x0 + RAD)
                    return mkap(_shp, [[ps, 128], [HC, ny], [1, nx]], off)

                def eview(m, y0, ny, x0, nx, _ex=ex):
                    ps = _ex.ap[0][0]
                    off = m * NEX + (y0 + RAD) * EC + (x0 + RAD)
                    return mkap(_ex, [[ps, 128], [EC, ny], [1, nx]], off)

                def cview(t, plane=0, nplane=1):
                    ps = t.ap[0][0]
                    return mkap(t, [[ps, 128], [1, nplane * NPIX]], plane * NPIX)

                # ---- loads (split across DMA queues; n3+z first) ----
                def shp_slice(lo, hi):
                    return mkap(
                        shp, [[shp.ap[0][0], 128], [1, (hi - lo) * NSH]], lo * NSH
                    )

                def src_slice(lo, hi):
                    base = shp_t.ap()[T]
                    return mkap(
                        base, [[7 * NSH, 128], [1, (hi - lo) * NSH]],
                        lo * NSH,
                    )

                nc.sync.dma_start(shp_slice(0, 3), src_slice(0, 3))      # nrm
                nc.sync.dma_start(shp_slice(6, 7), src_slice(6, 7))      # z
                nc.scalar.dma_start(shp_slice(3, 6), src_slice(3, 6))    # col
                nc.gpsimd.dma_start(cview(dl3), dz_t.ap()[T])            # dz
                nc.vector.reciprocal(cview(recip), cview(dl3))
                nc.vector.memset(cview(acc, 0, 4), 0.0)

                def accum_col_w(sdy, sdx, w_tile, _shp=shp, _acc=acc, _dl3=dl3):
                    """acc[0:3] += col_shifted * w;  acc[3] += w"""
                    wv = cview(w_tile)
                    if USE_BCAST:
                        ps = _shp.ap[0][0]
                        col3 = mkap(
                            _shp,
                            [[ps, 128], [NSH, 3], [HC, RB], [1, CW]],
                            PCR * NSH + (sdy + RAD) * HC + (sdx + RAD),
                        )
                        wb = mkap(
                            w_tile,
                            [[w_tile.ap[0][0], 128], [0, 3], [CW, RB], [1, CW]],
                            0,
                        )
                        d3 = mkap(
                            _dl3,
                            [[_dl3.ap[0][0], 128], [NPIX, 3], [CW, RB], [1, CW]],
                            0,
                        )
                        nc.vector.tensor_tensor(d3, col3, wb, Alu.mult)
                    else:
                        for ch in range(3):
                            nc.vector.tensor_tensor(
                                cview(_dl3, ch),
                                hview(PCR + ch, sdy, RB, sdx, CW),
                                wv,
                                Alu.mult,
                            )
                    a3 = cview(_acc, 0, 3)
                    nc.vector.tensor_tensor(a3, a3, cview(_dl3, 0, 3), Alu.add)
                    aw = cview(_acc, 3)
                    nc.vector.tensor_tensor(aw, aw, wv, Alu.add)

                def do_tap(sdy, sdx, lnd_view, w_tile, _rp=rprime):
                    """accumulate one tap (shift sdy,sdx) given its ln(dot) view"""
                    uas = rot_pool.tile([128, NPIX], f32, tag="uas")
                    u = cview(uas)
                    nc.vector.tensor_tensor(
                        u,
                        hview(PZ, sdy, RB, sdx, CW),
                        hview(PZ, 0, RB, 0, CW),
                        Alu.subtract,
                    )
                    nc.vector.tensor_tensor(u, u, cview(_rp), Alu.mult)
                    nc.scalar.activation(u, u, Act.Abs)
                    nc.vector.scalar_tensor_tensor(
                        u, u, -1.0, lnd_view, Alu.mult, Alu.add
                    )
                    d2 = sdy * sdy + sdx * sdx
                    nc.scalar.activation(
                        cview(w_tile), u, Act.Exp, bias=-(d2) / 8.0, scale=128.0
                    )
                    accum_col_w(sdy, sdx, w_tile)

                # ---- center tap: w = exp(128*ln(clip(|n|^2))) ----
                dote = eview(0, 0, RB, 0, CW)
                ptmp = eview(1, 0, RB, 0, CW)
                lnde = eview(2, 0, RB, 0, CW)
                for k in range(3):
                    nv = hview(PNX + k, 0, RB, 0, CW)
                    if k == 0:
                        nc.vector.tensor_tensor(dote, nv, nv, Alu.mult)
                    else:
                        nc.vector.tensor_tensor(ptmp, nv, nv, Alu.mult)
                        nc.vector.tensor_tensor(dote, dote, ptmp, Alu.add)
                nc.vector.tensor_scalar(dote, dote, CLIP_LO, 1.0, Alu.max, Alu.min)
                nc.scalar.activation(lnde, dote, Act.Ln)
                wt = psum_pool.tile([128, NPIX], f32, tag="w")
                nc.scalar.activation(cview(wt), lnde, Act.Exp, bias=0.0, scale=128.0)
                accum_col_w(0, 0, wt)

                # ---- the 60 +/- pairs, grouped by distance ----
                cur_d2 = None
                for dy, dx in taps:
                    d2 = dy * dy + dx * dx
                    if d2 != cur_d2:
                        cur_d2 = d2
                        invd = 1.0 / (128.0 * math.sqrt(d2))
                        nc.vector.tensor_scalar(
                            cview(rprime), cview(recip), invd, RCAP, Alu.mult, Alu.min
                        )
                    # expanded region: py in [-dy, RB), px in [xlo, xlo+XW)
                    ylo = -dy
                    NY = RB + dy
                    xlo = -dx if dx > 0 else 0
                    XW = CW + abs(dx)
                    if XW % 2 and abs(dx) < RAD:
                        XW += 1  # even innermost dim lets the clip TS run 2x
                    dote = eview(0, ylo, NY, xlo, XW)
                    ptmp = eview(1, ylo, NY, xlo, XW)
                    lnde_full = eview(2, ylo, NY, xlo, XW)
                    for k in range(3):
                        cenv = hview(PNX + k, ylo, NY, xlo, XW)
                        shv = hview(PNX + k, ylo + dy, NY, xlo + dx, XW)
                        if k == 0:
                            nc.vector.tensor_tensor(dote, cenv, shv, Alu.mult)
                        else:
                            nc.vector.tensor_tensor(ptmp, cenv, shv, Alu.mult)
                            nc.vector.tensor_tensor(dote, dote, ptmp, Alu.add)
                    nc.vector.tensor_scalar(
                        dote, dote, CLIP_LO, 1.0, Alu.max, Alu.min
                    )
                    nc.scalar.activation(lnde_full, dote, Act.Ln)
                    # tap +t reads lnd at (0,0); tap -t reads lnd at (-dy,-dx)
                    wt = psum_pool.tile([128, NPIX], f32, tag="w")
                    do_tap(dy, dx, eview(2, 0, RB, 0, CW), wt)
                    wt = psum_pool.tile([128, NPIX], f32, tag="w")
                    do_tap(-dy, -dx, eview(2, -dy, RB, -dx, CW), wt)

                # ---- epilogue: out = acc_c / acc_w ----
                inv = cview(rprime)
                nc.vector.reciprocal(inv, cview(acc, 3))
                if USE_BCAST:
                    ob = mkap(
                        dl3,
                        [[dl3.ap[0][0], 128], [NPIX, 3], [CW, RB], [1, CW]],
                        0,
                    )
                    invb = mkap(
                        rprime,
                        [[rprime.ap[0][0], 128], [0, 3], [CW, RB], [1, CW]],
                        0,
                    )
                    accb = mkap(
                        acc,
                        [[acc.ap[0][0], 128], [NPIX, 3], [CW, RB], [1, CW]],
                        0,
                    )
                    nc.vector.tensor_tensor(ob, accb, invb, Alu.mult)
                else:
                    for ch in range(3):
                        nc.vector.tensor_tensor(
                            cview(dl3, ch), cview(acc, ch), inv, Alu.mult
                        )
                nc.sync.dma_start(out_t.ap()[T], dl3)
    _split_multi_waits(nc)
    return nc


def _prep_inputs(col, nrm, zdz, ntiles=NTILES):
    from numpy.lib.stride_tricks import sliding_window_view

    shp_in = np.empty((NCORES, ntiles, 128, 7 * NSH), np.float32)
    dz_in = np.empty((NCORES, ntiles, 128, NPIX), np.float32)
    rstarts = RB * np.arange(GR)
    cstarts = CW * np.arange(GC)
    for c in range(NCORES):
        b, h = divmod(c, 2)
        planes = np.zeros((8, PR, PC), np.float32)
        planes[6] = ZPAD
        gxs = max(0, 512 * h - RAD)
        gxe = min(W, 512 * h + COLS + RAD)
        js = gxs - (512 * h - RAD)
        je = js + (gxe - gxs)
        sl = np.s_[RAD : RAD + H, js:je]
        for k in range(3):
            planes[PNX + k][sl] = nrm[b, :, gxs:gxe, k]
            planes[PCR + k][sl] = col[b, :, gxs:gxe, k]
        planes[PZ][sl] = zdz[b, :, gxs:gxe, 0]
        planes[7][sl] = zdz[b, :, gxs:gxe, 1]
        for k in range(7):
            sw = sliding_window_view(planes[k], (HB, HC))
            for T in range(ntiles):
                blk = sw[512 * T + rstarts][:, cstarts]  # [GR, GC, HB, HC]
                shp_in[c, T, :, k * NSH : (k + 1) * NSH] = blk.reshape(128, NSH)
        dzc = planes[7, RAD : RAD + H, RAD : RAD + COLS]
        for T in range(ntiles):
            d = dzc[512 * T : 512 * (T + 1)].reshape(GR, RB, GC, CW)
            dz_in[c, T] = d.transpose(0, 2, 1, 3).reshape(128, NPIX)
    return shp_in, dz_in


def _gather_output(results, ntiles=NTILES):
    out = np.empty((B, H, W, 3), np.float32)
    for c in range(NCORES):
        b, h = divmod(c, 2)
        o = results[c]["out"]  # [ntiles, 128, 3*NPIX]
        o = o.reshape(ntiles, GR, GC, 3, RB, CW)
        img = o.transpose(3, 0, 1, 4, 2, 5).reshape(3, ntiles * 512, COLS)
        out[b, : ntiles * 512, 512 * h : 512 * h + COLS, :] = np.moveaxis(img, 0, -1)
    return out


def kernel(col, nrm, zdz):
    from concourse import bass_utils

    if "nc" not in _cache:
        _cache["nc"] = _build_program()
    nc = _cache["nc"]
    shp_in, dz_in = _prep_inputs(
        np.asarray(col, np.float32), np.asarray(nrm, np.float32),
        np.asarray(zdz, np.float32)
    )
    in_maps = [{"shp": shp_in[c], "dz": dz_in[c]} for c in range(NCORES)]
    res = bass_utils.run_bass_kernel_spmd(nc, in_maps, core_ids=list(range(NCORES)))
    return _gather_output(res.results)


# revision 15
# speedup vs baseline: 1.0146x; 1.0146x over previous
"""Bilateral denoiser (11x11 window) on 8 Trainium2 NeuronCores.

Sharding: 8 cores = 4 images x 2 column halves (512 cols each, full 1024 rows).
Device layout: each of 128 partitions owns a 32x64 pixel block with a 5-pixel
halo stored in the free dimension, so both dy and dx taps are free-dim offsets.
Two mega-tiles per core cover rows [0,512) and [512,1024).  The host
pre-gathers each partition's halo'd block so all DMAs are contiguous 2D.

Weight math per tap t=(dy,dx), d=|t|:
  w_ref = exp(-d^2/8) * clip(n.t_n,0,1)^128 * exp(-|t_z-z|/max(dz*d,1e-4))
computed as   s = ln(clip(dot,1e-30,1)) - |(t_z-z) * r'|,  w = exp(128*s - d^2/8)
with r' = min(recip_dz/(128 d), 1e4/128).  Mirror taps (-t) reuse ln(dot) of
(+t) shifted, since dot is symmetric.  Out-of-image pixels are padded
host-side with z=1e18 (forces w=0) and 0 for the other planes.
"""

import sys

for _p in ("/root/.axon_site", "/root/.axon_site/_ro/trn_rl_repo",
           "/root/.axon_site/_ro/pypackages", "/opt/trn_rl_repo", "/opt/pypackages"):
    if _p not in sys.path:
        sys.path.append(_p)

import math
import numpy as np

B, H, W = 4, 1024, 1024
RAD = 5
NCORES = 8
COLS = 512          # cols per core
RB, CW = 32, 64     # block rows/cols per partition
HB, HC = RB + 2 * RAD, CW + 2 * RAD          # 42, 74  halo'd block
EB, EC = RB + RAD, CW + 2 * RAD              # 37, 74  expanded (dot/ln) frame
NTILES = 2          # mega-tiles (rows 0..512, 512..1024)
GR, GC = 16, 8      # partition grid: 16 row-blocks x 8 col-blocks
PR, PC = H + 2 * RAD, COLS + 2 * RAD         # 1034 x 522 padded plane dims
ZPAD = 1.0e18
CLIP_LO = 1.0e-30
RCAP = 1.0e4 / 128.0
USE_BCAST = True  # step-0 broadcast APs on DVE inputs
NPIX = RB * CW      # 2048
NSH = HB * HC       # 3108 halo'd plane elems/partition
NEX = EB * EC       # 2738 expanded plane elems/partition

# plane order in the shifted-plane bundle
PNX, PNY, PNZ, PCR, PCG, PCB, PZ = range(7)

_cache = {}


def _positive_half_taps():
    taps = []
    for dy in range(0, RAD + 1):
        for dx in range(-RAD, RAD + 1):
            if dy == 0 and dx <= 0:
                continue
            taps.append((dy, dx))
    # group by d^2 so rprime is computed once per distinct distance
    taps.sort(key=lambda t: (t[0] * t[0] + t[1] * t[1], t))
    return taps


def _build_program(ntiles=NTILES, npairs=None):
    import concourse.bass as bass
    import concourse.mybir as mybir
    import concourse.tile as tile

    f32 = mybir.dt.float32
    Alu = mybir.AluOpType
    Act = mybir.ActivationFunctionType

    # --- workaround: this walrus build rejects >1 sync-wait per TPB_CTRL ---
    def _patched_drain_and_barrier(self, tick_clock, wait_clock):
        from bass_rust import ScopedClock

        probe = mybir.InstDrain(
            name="wait-probe", engine=mybir.EngineType.SP, ins=[], outs=[]
        )
        wait_clock.add_sem_waits(probe, ScopedClock({None: tick_clock.global_clock}))
        si = probe.sync_info
        waits = list(si.on_wait) if si is not None else []
        handles = {h.num: h for h in self.sems.allocated().values()}
        for wt in waits:
            assert wt.wait_reg is None and wt.wait_mode == "sem-ge-imm", wt
            self.nc.sync.wait_ge(handles[wt.id], wt.wait_value)
        self.nc.sync.drain()
        self.nc.all_engine_barrier()
        popped = self.nc._tile_sem_poison_stack.pop()
        assert popped is self._sem_poison
        self.nc.clear_and_free_semaphores(list(self.sems.allocated().values()))
        self.nc.all_engine_barrier()

    tile.TileContext._drain_and_barrier = _patched_drain_and_barrier

    def _split_multi_waits(nc):
        # This walrus build accepts only one sync-wait per instruction;
        # hoist extra waits onto NoOps inserted just before, same engine.
        total = 0
        for blk in nc.main_func.blocks:
            il = blk.instructions
            k = 0
            while k < len(il):
                inst = il[k]
                si = inst.sync_info
                if si is not None and si.on_wait and len(si.on_wait) > 1:
                    waits = list(si.on_wait)
                    for j, wt in enumerate(waits[:-1]):
                        nop = mybir.InstNoOp(
                            name=f"{inst.name}-hw{j}",
                            engine=inst.engine,
                            ins=[],
                            outs=[],
                        )
                        nop.sync_info = mybir.SyncInfo(on_wait=[wt], on_update=[])
                        nc.register_instruction(nop, overwrite=True)
                        il.insert(k, nop)
                        k += 1
                    inst.sync_info = mybir.SyncInfo(
                        on_wait=[waits[-1]], on_update=list(si.on_update or [])
                    )
                    total += 1
                k += 1
        return total

    nc = bass.Bass("TRN2")

    taps = _positive_half_taps()
    if npairs is not None:
        taps = taps[:npairs]
    dsqs = sorted({dy * dy + dx * dx for dy, dx in taps})
    for d2 in dsqs:
        v = -(d2) / 8.0
        key = (f32, float(v))
        if key not in nc.const_aps.aps:
            t = nc.alloc_sbuf_tensor(f"cbias_{d2}", [128, 1], f32)
            nc.gpsimd.memset(t.ap(), float(v))
            nc.const_aps.aps[key] = t.ap()
    nc.all_engine_barrier()

    shp_t = nc.dram_tensor("shp", [ntiles, 128, 7 * NSH], f32, kind="ExternalInput")
    dz_t = nc.dram_tensor("dz", [ntiles, 128, NPIX], f32, kind="ExternalInput")
    out_t = nc.dram_tensor("out", [ntiles, 128, 3 * NPIX], f32, kind="ExternalOutput")

    def mkap(base, dims, extra_off):
        a = base.copy()
        a.ap = mybir.VecI64Pair(dims)
        a.offset = a.offset + extra_off
        return a

    with tile.TileContext(nc) as tc:
        with tc.tile_pool(name="shp", bufs=1) as shp_pool, \
             tc.tile_pool(name="exp", bufs=1) as exp_pool, \
             tc.tile_pool(name="cen", bufs=1) as cen_pool, \
             tc.tile_pool(name="rot", bufs=2) as rot_pool, \
             tc.tile_pool(name="pw", bufs=2, space="PSUM") as psum_pool:
            for T in range(ntiles):
                shp = shp_pool.tile([128, 7 * NSH], f32, tag="shp")
                ex = exp_pool.tile([128, 3 * NEX], f32, tag="ex")
                acc = cen_pool.tile([128, 4 * NPIX], f32, tag="acc")
                recip = cen_pool.tile([128, NPIX], f32, tag="recip")
                rprime = cen_pool.tile([128, NPIX], f32, tag="rprime")
                dl3 = cen_pool.tile([128, 3 * NPIX], f32, tag="dl3")

                def hview(k, y0, ny, x0, nx, _shp=shp):
                    # halo'd plane k at block coords rows y0.., cols x0..
                    # (block-local pixel coords; -5 = first halo row/col)
                    ps = _shp.ap[0][0]
                    off = k * NSH + (y0 + RAD) * HC + (x0 + RAD)
                    return mkap(_shp, [[ps, 128], [HC, ny], [1, nx]], off)

                def eview(m, y0, ny, x0, nx, _ex=ex):
                    ps = _ex.ap[0][0]
                    off = m * NEX + (y0 + RAD) * EC + (x0 + RAD)
                    return mkap(_ex, [[ps, 128], [EC, ny], [1, nx]], off)

                def cview(t, plane=0, nplane=1):
                    ps = t.ap[0][0]
                    return mkap(t, [[ps, 128], [1, nplane * NPIX]], plane * NPIX)

                # ---- loads (split across DMA queues; n3+z first) ----
                def shp_slice(lo, hi):
                    return mkap(
                        shp, [[shp.ap[0][0], 128], [1, (hi - lo) * NSH]], lo * NSH
                    )

                def src_slice(lo, hi):
                    base = shp_t.ap()[T]
                    return mkap(
                        base, [[7 * NSH, 128], [1, (hi - lo) * NSH]],
                        lo * NSH,
                    )

                nc.sync.dma_start(shp_slice(0, 3), src_slice(0, 3))      # nrm
                nc.sync.dma_start(shp_slice(6, 7), src_slice(6, 7))      # z
                nc.scalar.dma_start(shp_slice(3, 6), src_slice(3, 6))    # col
                nc.gpsimd.dma_start(cview(dl3), dz_t.ap()[T])            # dz
                nc.vector.reciprocal(cview(recip), cview(dl3))
                nc.vector.memset(cview(acc, 0, 4), 0.0)

                def accum_col_w(sdy, sdx, w_tile, _shp=shp, _acc=acc, _dl3=dl3):
                    """acc[0:3] += col_shifted * w;  acc[3] += w"""
                    wv = cview(w_tile)
                    if USE_BCAST:
                        ps = _shp.ap[0][0]
                        col3 = mkap(
                            _shp,
                            [[ps, 128], [NSH, 3], [HC, RB], [1, CW]],
                            PCR * NSH + (sdy + RAD) * HC + (sdx + RAD),
                        )
                        wb = mkap(
                            w_tile,
                            [[w_tile.ap[0][0], 128], [0, 3], [CW, RB], [1, CW]],
                            0,
                        )
                        d3 = mkap(
                            _dl3,
                            [[_dl3.ap[0][0], 128], [NPIX, 3], [CW, RB], [1, CW]],
                            0,
                        )
                        nc.vector.tensor_tensor(d3, col3, wb, Alu.mult)
                    else:
                        for ch in range(3):
                            nc.vector.tensor_tensor(
                                cview(_dl3, ch),
                                hview(PCR + ch, sdy, RB, sdx, CW),
                                wv,
                                Alu.mult,
                            )
                    a3 = cview(_acc, 0, 3)
                    nc.vector.tensor_tensor(a3, a3, cview(_dl3, 0, 3), Alu.add)
                    aw = cview(_acc, 3)
                    nc.vector.tensor_tensor(aw, aw, wv, Alu.add)

                def do_tap(sdy, sdx, lnd_view, adz_view, w_tile, _rp=rprime):
                    """accumulate one tap given its ln(dot) and |dz| views"""
                    uas = rot_pool.tile([128, NPIX], f32, tag="uas")
                    u = cview(uas)
                    nc.vector.tensor_tensor(u, adz_view, cview(_rp), Alu.mult)
                    nc.vector.scalar_tensor_tensor(
                        u, u, -1.0, lnd_view, Alu.mult, Alu.add
                    )
                    d2 = sdy * sdy + sdx * sdx
                    nc.scalar.activation(
                        cview(w_tile), u, Act.Exp, bias=-(d2) / 8.0, scale=128.0
                    )
                    accum_col_w(sdy, sdx, w_tile)

                # ---- center tap: w = exp(128*ln(clip(|n|^2))) ----
                dote = eview(0, 0, RB, 0, CW)
                ptmp = eview(1, 0, RB, 0, CW)
                lnde = eview(2, 0, RB, 0, CW)
                for k in range(3):
                    nv = hview(PNX + k, 0, RB, 0, CW)
                    if k == 0:
                        nc.vector.tensor_tensor(dote, nv, nv, Alu.mult)
                    else:
                        nc.vector.tensor_tensor(ptmp, nv, nv, Alu.mult)
                        nc.vector.tensor_tensor(dote, dote, ptmp, Alu.add)
                nc.vector.tensor_scalar(dote, dote, CLIP_LO, 1.0, Alu.max, Alu.min)
                nc.scalar.activation(lnde, dote, Act.Ln)
                wt = psum_pool.tile([128, NPIX], f32, tag="w")
                nc.scalar.activation(cview(wt), lnde, Act.Exp, bias=0.0, scale=128.0)
                accum_col_w(0, 0, wt)

                # ---- the 60 +/- pairs, grouped by distance ----
                cur_d2 = None
                for dy, dx in taps:
                    d2 = dy * dy + dx * dx
                    if d2 != cur_d2:
                        cur_d2 = d2
                        invd = 1.0 / (128.0 * math.sqrt(d2))
                        nc.vector.tensor_scalar(
                            cview(rprime), cview(recip), invd, RCAP, Alu.mult, Alu.min
                        )
                    # expanded region: py in [-dy, RB), px in [xlo, xlo+XW)
                    ylo = -dy
                    NY = RB + dy
                    xlo = -dx if dx > 0 else 0
                    XW = CW + abs(dx)
                    if XW % 2 and abs(dx) < RAD:
                        XW += 1  # even innermost dim lets the clip TS run 2x
                    dote = eview(0, ylo, NY, xlo, XW)
                    ptmp = eview(1, ylo, NY, xlo, XW)
                    lnde_full = eview(2, ylo, NY, xlo, XW)
                    for k in range(3):
                        cenv = hview(PNX + k, ylo, NY, xlo, XW)
                        shv = hview(PNX + k, ylo + dy, NY, xlo + dx, XW)
                        if k == 0:
                            nc.vector.tensor_tensor(dote, cenv, shv, Alu.mult)
                        else:
                            nc.vector.tensor_tensor(ptmp, cenv, shv, Alu.mult)
                            nc.vector.tensor_tensor(dote, dote, ptmp, Alu.add)
                    nc.vector.tensor_scalar(
                        dote, dote, CLIP_LO, 1.0, Alu.max, Alu.min
                    )
                    nc.scalar.activation(lnde_full, dote, Act.Ln)
                    # |dz| is shift-antisymmetric: |z(p-t)-z(p)| = adz(p-t).
                    # Compute once per pair on the expanded frame, in the dead
                    # ptmp plane (no WAR with the ln read of dote).
                    nc.vector.tensor_tensor(
                        ptmp,
                        hview(PZ, ylo + dy, NY, xlo + dx, XW),
                        hview(PZ, ylo, NY, xlo, XW),
                        Alu.subtract,
                    )
                    nc.scalar.activation(ptmp, ptmp, Act.Abs)
                    # tap +t reads lnd/adz at (0,0); tap -t at (-dy,-dx)
                    wt = psum_pool.tile([128, NPIX], f32, tag="w")
                    do_tap(
                        dy, dx,
                        eview(2, 0, RB, 0, CW), eview(1, 0, RB, 0, CW), wt,
                    )
                    wt = psum_pool.tile([128, NPIX], f32, tag="w")
                    do_tap(
                        -dy, -dx,
                        eview(2, -dy, RB, -dx, CW), eview(1, -dy, RB, -dx, CW), wt,
                    )

                # ---- epilogue: out = acc_c / acc_w ----
                inv = cview(rprime)
                nc.vector.reciprocal(inv, cview(acc, 3))
                if USE_BCAST:
                    ob = mkap(
                        dl3,
                        [[dl3.ap[0][0], 128], [NPIX, 3], [CW, RB], [1, CW]],
                        0,
                    )
                    invb = mkap(
                        rprime,
                        [[rprime.ap[0][0], 128], [0, 3], [CW, RB], [1, CW]],
                        0,
                    )
                    accb = mkap(
                        acc,
                        [[acc.ap[0][0], 128], [NPIX, 3], [CW, RB], [1, CW]],
                        0,
                    )
                    nc.vector.tensor_tensor(ob, accb, invb, Alu.mult)
                else:
                    for ch in range(3):
                        nc.vector.tensor_tensor(
                            cview(dl3, ch), cview(acc, ch), inv, Alu.mult
                        )
                nc.sync.dma_start(out_t.ap()[T], dl3)
    _split_multi_waits(nc)
    return nc


def _prep_inputs(col, nrm, zdz, ntiles=NTILES):
    from numpy.lib.stride_tricks import sliding_window_view

    shp_in = np.empty((NCORES, ntiles, 128, 7 * NSH), np.float32)
    dz_in = np.empty((NCORES, ntiles, 128, NPIX), np.float32)
    rstarts = RB * np.arange(GR)
    cstarts = CW * np.arange(GC)
    for c in range(NCORES):
        b, h = divmod(c, 2)
        planes = np.zeros((8, PR, PC), np.float32)
        planes[6] = ZPAD
        gxs = max(0, 512 * h - RAD)
        gxe = min(W, 512 * h + COLS + RAD)
        js = gxs - (512 * h - RAD)
        je = js + (gxe - gxs)
        sl = np.s_[RAD : RAD + H, js:je]
        for k in range(3):
            planes[PNX + k][sl] = nrm[b, :, gxs:gxe, k]
            planes[PCR + k][sl] = col[b, :, gxs:gxe, k]
        planes[PZ][sl] = zdz[b, :, gxs:gxe, 0]
        planes[7][sl] = zdz[b, :, gxs:gxe, 1]
        for k in range(7):
            sw = sliding_window_view(planes[k], (HB, HC))
            for T in range(ntiles):
                blk = sw[512 * T + rstarts][:, cstarts]  # [GR, GC, HB, HC]
                shp_in[c, T, :, k * NSH : (k + 1) * NSH] = blk.reshape(128, NSH)
        dzc = planes[7, RAD : RAD + H, RAD : RAD + COLS]
        for T in range(ntiles):
            d = dzc[512 * T : 512 * (T + 1)].reshape(GR, RB, GC, CW)
            dz_in[c, T] = d.transpose(0, 2, 1, 3).reshape(128, NPIX)
    return shp_in, dz_in


def _gather_output(results, ntiles=NTILES):
    out = np.empty((B, H, W, 3), np.float32)
    for c in range(NCORES):
        b, h = divmod(c, 2)
        o = results[c]["out"]  # [ntiles, 128, 3*NPIX]
        o = o.reshape(ntiles, GR, GC, 3, RB, CW)
        img = o.transpose(3, 0, 1, 4, 2, 5).reshape(3, ntiles * 512, COLS)
        out[b, : ntiles * 512, 512 * h : 512 * h + COLS, :] = np.moveaxis(img, 0, -1)
    return out


def kernel(col, nrm, zdz):
    from concourse import bass_utils

    if "nc" not in _cache:
        _cache["nc"] = _build_program()
    nc = _cache["nc"]
    shp_in, dz_in = _prep_inputs(
        np.asarray(col, np.float32), np.asarray(nrm, np.float32),
        np.asarray(zdz, np.float32)
    )
    in_maps = [{"shp": shp_in[c], "dz": dz_in[c]} for c in range(NCORES)]
    res = bass_utils.run_bass_kernel_spmd(nc, in_maps, core_ids=list(range(NCORES)))
    return _gather_output(res.results)
